# revision 29
# baseline (speedup 1.0000x reference)
"""Trainium2 Bass kernel for a fused multi-head attention block.

Reference computation (B=4, T=2048, D=1152, H=8, HD=144, full rotary):
    q,k,v = x@Wq.T, x@Wk.T, x@Wv.T   (per head)
    q,k   = rope(q, k, cos, sin)
    o     = softmax(q k^T / sqrt(HD)) v
    out   = o @ Wo.T
Sharding (8 cores): core c = (batch b = c//2, head-group hg = c%2).
Each core computes 4 heads of one batch and a partial output
out_part = o_local @ Wo[:, hg_cols].T ; host sums the two partials per batch.

Per-core layout decisions:
  * Host passes x transposed (xT [D, T]) and weights pre-transposed so that
    every matmul contraction sits on the partition axis.
  * q/k head dims are padded 144 -> 160 and reordered on the host into
    [h0:0-127 | h1:0-127 | h2:0-127 | h3:0-127 | b-block 4x(16 real + 16 zero)]
    so that per-head tiles stay 128/32-partition aligned on chip.
  * Scores are computed transposed (S^T [keys, q]) so the PV matmul needs no
    transpose, and the softmax denominator comes free by appending a ones
    column to v (o_psum[:, 144] = sum(exp(S))).
  * exp() has no max-subtraction: scores*scale have std ~0.7, |S|<6, safely
    inside fp32/bf16 exp range.
  * Phase B pipelines kt-pairs: scores land in one of three 2-bank PSUM
    sets, one wide [128,1024] exp per set (amortizes ACT overhead and
    decouples the in-order PE queue from the scalar engine), then 8 PV
    matmuls per pair at the PE's native issue cadence.
  * dtypes: projections in bf16, attention matmuls bf16, final projection
    bf16 (FWL-eligible), all accumulation fp32 in PSUM.
"""

import numpy as np

B, T, D, H = 4, 2048, 1152, 8
HL = 4              # heads per core
HD = 144            # head dim
EP = 576            # q/k projection width: 4*128 + 4*16 (b-block packed, no pad)
DV = HL * HD        # 576, v/o width
NT = T // 128       # 16 t-tiles
KC = D // 128       # 9 contraction chunks
SCALE = float(HD) ** -0.5
NCORES = 8
GS = 2              # score key-tiles per PSUM set (2 banks)

_NC_CACHE = {}


def _build(debug=False):
    import concourse.bacc as bacc
    import concourse.mybir as mybir
    from concourse.tile import TileContext

    dt = mybir.dt
    f32, bf16 = dt.float32, dt.bfloat16
    AF = mybir.ActivationFunctionType

    nc = bacc.Bacc(
        "TRN2",
        target_bir_lowering=False,
        debug=debug,
        enable_asserts=False,
        num_devices=NCORES,
    )

    xT = nc.declare_dram_parameter("xT", [D, T], bf16, isOutput=False)
    wqT = nc.declare_dram_parameter("wqT", [D, EP], bf16, isOutput=False)
    wkT = nc.declare_dram_parameter("wkT", [D, EP], bf16, isOutput=False)
    wvT = nc.declare_dram_parameter("wvT", [D, DV], bf16, isOutput=False)
    woT = nc.declare_dram_parameter("woT", [DV, D], bf16, isOutput=False)
    cosN = nc.declare_dram_parameter("cosN", [T, HD], bf16, isOutput=False)
    identB = nc.declare_dram_parameter("identB", [128, 128], bf16, isOutput=False)
    sinN = nc.declare_dram_parameter("sinN", [T, HD], bf16, isOutput=False)
    out = nc.declare_dram_parameter("out", [T, D], f32, isOutput=True)

    def rope(qraw, qtl, cos3, sin3, tmps):
        """qraw [128, EP] f32 -> qtl [128, EP] bf16 with rotary applied.

        Column map: head h dim e<128 -> col 128h+e ; dim 128+j -> col 512+16h+j.
        rot_half partner: e<72 -> e+72 (sign -), e>=72 -> e-72 (sign +).
        cos3/sin3: [128, 4(bcast), 144] broadcast views of this t-tile's rows.
        Two full products m1=q*cos, m2=q*sin (4 ops), then 4 region combines.
        """
        qa = qraw[:, 0:512].rearrange("p (h e) -> p h e", h=HL)
        qb = qraw[:, 512:EP].rearrange("p (h e) -> p h e", h=HL)
        oa = qtl[:, 0:512].rearrange("p (h e) -> p h e", h=HL)
        ob = qtl[:, 512:EP].rearrange("p (h e) -> p h e", h=HL)
        m1, m2 = tmps
        m1a = m1[:, 0:512].rearrange("p (h e) -> p h e", h=HL)
        m1b = m1[:, 512:EP].rearrange("p (h e) -> p h e", h=HL)
        m2a = m2[:, 0:512].rearrange("p (h e) -> p h e", h=HL)
        m2b = m2[:, 512:EP].rearrange("p (h e) -> p h e", h=HL)
        v = nc.vector
        v.tensor_mul(m1a[:, :, 0:128], qa[:, :, 0:128], cos3[:, :, 0:128])
        v.tensor_mul(m1b[:, :, 0:16], qb[:, :, 0:16], cos3[:, :, 128:144])
        # m2[j] = q[j] * sin[partner(j)] so combines read m2 at the partner col
        v.tensor_mul(m2a[:, :, 0:56], qa[:, :, 0:56], sin3[:, :, 72:128])
        v.tensor_mul(m2a[:, :, 56:72], qa[:, :, 56:72], sin3[:, :, 128:144])
        v.tensor_mul(m2a[:, :, 72:128], qa[:, :, 72:128], sin3[:, :, 0:56])
        v.tensor_mul(m2b[:, :, 0:16], qb[:, :, 0:16], sin3[:, :, 56:72])
        # e in [0,56):  out = m1[e] - m2[e+72]
        v.tensor_sub(oa[:, :, 0:56], m1a[:, :, 0:56], m2a[:, :, 72:128])
        # e in [56,72): partner lives in the b block
        v.tensor_sub(oa[:, :, 56:72], m1a[:, :, 56:72], m2b[:, :, 0:16])
        # e in [72,128): out = m1[e] + m2[e-72]
        v.tensor_add(oa[:, :, 72:128], m1a[:, :, 72:128], m2a[:, :, 0:56])
        # e in [128,144): out = m1b + m2[56:72]
        v.tensor_add(ob[:, :, 0:16], m1b[:, :, 0:16], m2a[:, :, 56:72])

    with TileContext(nc) as tc:
        with tc.tile_pool(name="persist", bufs=1) as P0:
            ident_bf = P0.tile([128, 128], bf16, name="ident_bf", tag="ident_bf")

            qTa = [
                P0.tile([128, T], bf16, name=f"qTa{h}", tag=f"qTa{h}")
                for h in range(HL)
            ]
            kTa = [
                P0.tile([128, T], bf16, name=f"kTa{h}", tag=f"kTa{h}")
                for h in range(HL)
            ]
            qTB = P0.tile([64, T], bf16, name="qTB", tag="qTB")
            kTB = P0.tile([64, T], bf16, name="kTB", tag="kTB")
            # per-head replicas of the b-block rows at all four 32-row groups,
            # so consecutive key-tiles' K=32 score matmuls can issue to
            # distinct PE row-groups and overlap in the array
            qTBr = [
                P0.tile([128, T], bf16, name=f"qTBr{h}", tag=f"qTBr{h}")
                for h in range(HL)
            ]
            kTBr = [
                P0.tile([128, T], bf16, name=f"kTBr{h}", tag=f"kTBr{h}")
                for h in range(HL)
            ]
            vt = [
                P0.tile([128, HL * (HD + 1)], bf16, name=f"v{t}", tag=f"v{t}")
                for t in range(NT)
            ]

            # ---------------- Phase A: projections + rope + transposes -----
            with (
                tc.tile_pool(name="pa", bufs=1) as pa,
                tc.tile_pool(name="paps", bufs=1, space="PSUM") as paps,
            ):
                xtiles = [
                    pa.tile([128, T], bf16, name=f"xTs{k}", tag=f"xTs{k}")
                    for k in range(KC)
                ]
                cos_sb = pa.tile([128, NT * HD], bf16, name="cos_sb", tag="cos_sb")
                sin_sb = pa.tile([128, NT * HD], bf16, name="sin_sb", tag="sin_sb")

                # PE warm-up: dummy transposes on a zeroed tile keep the PE
                # busy while the first x/w DMAs land, so HAM un-throttles
                # before the real matmuls start. memset needs no DMA, so the
                # PE can start within ~0.3us of kernel entry.
                warm_in = pa.tile([128, 128], bf16, name="warm_in", tag="warm_in")
                nc.vector.memset(warm_in[:], 0.0)
                for _ in range(40):
                    warm_ps = paps.tile([128, 128], bf16, name="tp", tag="tp", bufs=2)
                    nc.tensor.transpose(warm_ps[:], warm_in[:], warm_in[:])

                def trig3(sb, n):
                    # [128, 144] row block for t-tile n, broadcast over 4 heads
                    return (
                        sb[:, n * HD : (n + 1) * HD]
                        .rearrange("p (o r) -> p o r", o=1)
                        .to_broadcast([128, HL, HD])
                    )

                dma_engines = [nc.gpsimd, nc.scalar, nc.sync]

                def proj_phase(wdram, width, consume_head, consume_tail, first=False):
                    wtiles = []
                    for k in range(KC):
                        wt_ = pa.tile(
                            [128, EP], bf16, name=f"w{k}", tag=f"W{k}"
                        )
                        nsp = 2 if (first and k < 3) else 1
                        w_ = width // nsp
                        for j in range(nsp):
                            nc.sync.dma_start(
                                wt_[:, j * w_ : (j + 1) * w_],
                                wdram[k * 128 : (k + 1) * 128, j * w_ : (j + 1) * w_],
                            )
                        wtiles.append(wt_)
                    if first:
                        # x chunks in column-major order (chunk c of every k
                        # before chunk c+1) because the n-loop consumes x
                        # column-first across all k tiles; spread dispatches
                        # over three engine queues so the per-trigger cost
                        # (~0.6us) doesn't serialize the startup
                        for c in range(4):
                            for k in range(KC):
                                eng = dma_engines[(c * KC + k) % 3]
                                eng.dma_start(
                                    xtiles[k][:, c * 512 : (c + 1) * 512],
                                    xT[
                                        k * 128 : (k + 1) * 128,
                                        c * 512 : (c + 1) * 512,
                                    ],
                                )
                    half = width // 2
                    pending = None
                    for n in range(NT):
                        ps0 = paps.tile([128, 320], f32, name="ps0", tag="proj", bufs=6)
                        ps1 = paps.tile([128, 320], f32, name="ps1", tag="proj", bufs=6)
                        for k in range(KC):
                            st, sp = k == 0, k == KC - 1
                            lhs = xtiles[k][:, n * 128 : (n + 1) * 128]
                            nc.tensor.matmul(
                                ps0[:, 0:half],
                                lhs,
                                wtiles[k][:, 0:half],
                                start=st,
                                stop=sp,
                            )
                            nc.tensor.matmul(
                                ps1[:, 0:half],
                                lhs,
                                wtiles[k][:, half:width],
                                start=st,
                                stop=sp,
                            )
                        if pending is not None:
                            consume_tail(*pending)
                            pending = None
                        carry = consume_head(n, ps0[:, 0:half], ps1[:, 0:half])
                        if consume_tail is not None:
                            pending = (n, carry)
                    if pending is not None:
                        consume_tail(*pending)

                def qk_consume(qtl_dst_a, qtl_dst_b, repl_dst):
                    def head(n, ps0, ps1):
                        qraw = pa.tile([128, EP], f32, name="qraw", tag="qraw", bufs=3)
                        # last tiles' copies go to DVE so the scalar queue is
                        # fully drained when phase B's exps arrive (the exps
                        # otherwise sit ~10us behind the A-phase copy backlog)
                        if n >= NT - 4:
                            nc.vector.tensor_copy(qraw[:, 0 : EP // 2], ps0)
                            nc.vector.tensor_copy(qraw[:, EP // 2 : EP], ps1)
                        else:
                            nc.scalar.copy(qraw[:, 0 : EP // 2], ps0)
                            nc.scalar.copy(qraw[:, EP // 2 : EP], ps1)
                        qtl = pa.tile([128, EP], bf16, name="qtl", tag="qtl", bufs=3)
                        tA = pa.tile([128, EP], bf16, name="ropeA", tag="ropeA", bufs=2)
                        tB = pa.tile([128, EP], bf16, name="ropeB", tag="ropeB", bufs=2)
                        rope(qraw, qtl, trig3(cos_sb, n), trig3(sin_sb, n), (tA, tB))
                        return qtl

                    def tail(n, qtl):
                        for j in range(5):
                            rows = 128 if j < 4 else 64
                            tp = paps.tile(
                                [128, 128], bf16, name="tp", tag="tp", bufs=2
                            )
                            nc.tensor.transpose(
                                tp[0:rows, :],
                                qtl[:, 128 * j : 128 * j + rows],
                                ident_bf[:],
                            )
                            dst = qtl_dst_a[j] if j < 4 else qtl_dst_b
                            nc.any.tensor_copy(
                                dst[:, n * 128 : (n + 1) * 128], tp[0:rows, :]
                            )
                        # replicate the b-block by column halves as soon as the
                        # half is complete, so phase B's tail matmuls never
                        # wait on the end-of-phase rope/copy backlog
                        if n in (NT // 2 - 1, NT - 1):
                            c0 = 0 if n == NT // 2 - 1 else T // 2
                            cn = 0
                            for hh in range(HL):
                                for j in range(4):
                                    eng = dma_engines[cn % 3]
                                    eng.dma_start(
                                        repl_dst[hh][32 * j : 32 * j + 16, c0 : c0 + T // 2],
                                        qtl_dst_b[16 * hh : 16 * hh + 16, c0 : c0 + T // 2],
                                    )
                                    cn += 1

                    return head, tail

                def v_consume(n, ps0, ps1):
                    v3 = vt[n].rearrange("p (h e) -> p h e", h=HL)
                    nc.any.tensor_copy(
                        v3[:, 0:2, 0:HD],
                        ps0.rearrange("p (h e) -> p h e", h=2),
                    )
                    nc.any.tensor_copy(
                        v3[:, 2:4, 0:HD],
                        ps1.rearrange("p (h e) -> p h e", h=2),
                    )
                    nc.vector.memset(v3[:, :, HD : HD + 1], 1.0)

                qh, qt_ = qk_consume(qTa, qTB, qTBr)
                kh, kt_ = qk_consume(kTa, kTB, kTBr)
                proj_phase(wvT, DV, v_consume, None, first=True)
                nc.sync.dma_start(
                    cos_sb.rearrange("p (n r) -> p n r", n=NT),
                    cosN.rearrange("(n p) r -> p n r", p=128),
                )
                nc.sync.dma_start(
                    sin_sb.rearrange("p (n r) -> p n r", n=NT),
                    sinN.rearrange("(n p) r -> p n r", p=128),
                )
                # ident is only needed by the first transposes; issuing it
                # here keeps it from FIFO-blocking the v-phase loads on the
                # sync HWDGE ring (it is a slow 256B-line transfer)
                nc.scalar.dma_start(ident_bf[:], identB[:])
                # k before q: phase B's first blocks need ALL of k (kTa
                # columns + replicas) but only the first q tiles, so k's
                # end-of-phase rope/copy backlog drains during the q phase
                # while B's late blocks (qb=3) need the late q tiles only
                # long after they are ready
                proj_phase(wkT, EP, kh, kt_)
                proj_phase(wqT, EP, qh, qt_)

            # ---------------- Phase B: attention --------------------------
            with tc.tile_pool(name="pb", bufs=1) as pb:
                ot = [
                    pb.tile([128, DV], bf16, name=f"o{t}", tag=f"o{t}")
                    for t in range(NT)
                ]
                with tc.tile_pool(name="pbps", bufs=1, space="PSUM") as pbps:
                    NG = NT // GS
                    blocks = [(qb, h) for qb in range(4) for h in range(HL)]

                    def mk_block(qb, h):
                        # pack the 4 q-tile accumulators into 2 PSUM banks:
                        # 3*145 fp32 = 1740B fits one 2KB bank
                        o_ps3 = pbps.tile(
                            [128, 3 * (HD + 1)], f32, name="o_ps3", tag="o3", bufs=1
                        )
                        o_ps1 = pbps.tile(
                            [128, HD + 1], f32, name="o_ps1", tag="o1", bufs=1
                        )
                        o_ps = [
                            o_ps3[:, 0 : HD + 1],
                            o_ps3[:, HD + 1 : 2 * (HD + 1)],
                            o_ps3[:, 2 * (HD + 1) : 3 * (HD + 1)],
                            o_ps1[:],
                        ]

                        def s_exp_group(g):
                            # one 2-bank PSUM set per kt-pair: the K=128 head
                            # MMs, the K=16 tail MMs on rotating row-groups
                            # (concurrent in the PE array), then a single wide
                            # exp over the whole set (amortizes the ~150-cycle
                            # ACT start cost and halves semaphore traffic)
                            sps = pbps.tile(
                                [128, GS * 512], f32, name="sps", tag="sc", bufs=3
                            )
                            for j in range(GS):
                                kt = GS * g + j
                                nc.tensor.matmul(
                                    sps[:, j * 512 : (j + 1) * 512],
                                    kTa[h][:, kt * 128 : (kt + 1) * 128],
                                    qTa[h][:, qb * 512 : (qb + 1) * 512],
                                    start=True,
                                    stop=False,
                                )
                            for j in range(GS):
                                kt = GS * g + j
                                r = kt % 4
                                nc.tensor.matmul(
                                    sps[:, j * 512 : (j + 1) * 512],
                                    kTBr[h][32 * r : 32 * r + 16, kt * 128 : (kt + 1) * 128],
                                    qTBr[h][32 * r : 32 * r + 16, qb * 512 : (qb + 1) * 512],
                                    start=False,
                                    stop=True,
                                    tile_position=(32 * r, 0),
                                )
                            E = pb.tile(
                                [128, GS * 512], bf16, name="E", tag="E", bufs=6
                            )
                            nc.scalar.activation(E[:], sps[:], AF.Exp, scale=SCALE)
                            return E

                        def pv_group(g, E):
                            for j in range(GS):
                                kt = GS * g + j
                                for qt in range(4):
                                    # start/stop are bank-granular: qt 0-2
                                    # share o_ps3's bank, so only the
                                    # first/last bank write carries them
                                    if qt < 3:
                                        st = kt == 0 and qt == 0
                                        sp = kt == NT - 1 and qt == 2
                                    else:
                                        st = kt == 0
                                        sp = kt == NT - 1
                                    nc.tensor.matmul(
                                        o_ps[qt][:],
                                        E[:, j * 512 + qt * 128 : j * 512 + (qt + 1) * 128],
                                        vt[kt][:, (HD + 1) * h : (HD + 1) * (h + 1)],
                                        start=st,
                                        stop=sp,
                                    )

                        def readout():
                            for qt in range(4):
                                t = qb * 4 + qt
                                r = pb.tile([128, 1], f32, name="r", tag="r", bufs=4)
                                nc.vector.reciprocal(r[:], o_ps[qt][:, HD : HD + 1])
                                nc.vector.tensor_scalar_mul(
                                    ot[t][:, HD * h : HD * (h + 1)],
                                    o_ps[qt][:, 0:HD],
                                    r[:],
                                )

                        return s_exp_group, pv_group, readout

                    for qb, h in blocks:
                        s_exp_group, pv_group, readout = mk_block(qb, h)
                        Es = {0: s_exp_group(0), 1: s_exp_group(1)}
                        for g in range(NG):
                            if g + 2 < NG:
                                Es[g + 2] = s_exp_group(g + 2)
                            pv_group(g, Es.pop(g))
                        readout()

                # ---------------- Phase C: o^T + final projection ----------
                oTa = [
                    pb.tile([128, T], bf16, name=f"oTa{j}", tag=f"oTa{j}")
                    for j in range(4)
                ]
                oTb = pb.tile([64, T], bf16, name="oTb", tag="oTb")
                wo_tiles = []
                for k in range(5):
                    rows = 128 if k < 4 else 64
                    wot_ = pb.tile([128, D], bf16, name=f"wo{k}", tag=f"wo{k}")
                    nc.sync.dma_start(
                        wot_[0:rows, :], woT[k * 128 : k * 128 + rows, :]
                    )
                    wo_tiles.append(wot_)
                with tc.tile_pool(name="pcps", bufs=1, space="PSUM") as pcps:

                    def o_transp(t):
                        for j in range(4):
                            tp = pcps.tile(
                                [128, 128], bf16, name="tpo", tag="otp", bufs=3
                            )
                            nc.tensor.transpose(
                                tp[:],
                                ot[t][:, 128 * j : 128 * (j + 1)],
                                ident_bf[:],
                            )
                            nc.any.tensor_copy(
                                oTa[j][:, t * 128 : (t + 1) * 128], tp[:]
                            )
                        tpb = pcps.tile([64, 128], bf16, name="tpb", tag="otp", bufs=3)
                        nc.tensor.transpose(
                            tpb[:],
                            ot[t][:, 512:DV],
                            ident_bf[:],
                        )
                        nc.any.tensor_copy(
                            oTb[:, t * 128 : (t + 1) * 128], tpb[:]
                        )

                    def final(t):
                        fout = pb.tile(
                            [128, D], f32, name="fout", tag="fout", bufs=3
                        )
                        for j3 in range(3):
                            fps = pcps.tile([128, 384], f32, name="fps", tag="f", bufs=3)
                            for k in range(5):
                                lhs = (
                                    oTa[k][:, t * 128 : (t + 1) * 128]
                                    if k < 4
                                    else oTb[:, t * 128 : (t + 1) * 128]
                                )
                                nc.tensor.matmul(
                                    fps[:],
                                    lhs,
                                    wo_tiles[k][
                                        0 : (128 if k < 4 else 64),
                                        384 * j3 : 384 * (j3 + 1),
                                    ],
                                    start=(k == 0),
                                    stop=(k == 4),
                                )
                            nc.any.tensor_copy(
                                fout[:, 384 * j3 : 384 * (j3 + 1)], fps[:]
                            )
                        # one big store per tile, alternating the two HWDGE
                        # rings (SWDGE/gpsimd has a slow end-of-kernel drain):
                        # fewer DMAs means the per-transfer HBM-write receipt
                        # latency doesn't serialize into a multi-us tail
                        out_eng = [nc.sync, nc.scalar][t % 2]
                        out_eng.dma_start(
                            out[t * 128 : (t + 1) * 128, :], fout[:]
                        )

                    o_transp(0)
                    for t in range(NT):
                        if t + 1 < NT:
                            o_transp(t + 1)
                        final(t)

    nc.compile()
    return nc


def get_nc(debug=False):
    key = bool(debug)
    if key not in _NC_CACHE:
        _NC_CACHE[key] = _build(debug)
    return _NC_CACHE[key]


def make_in_maps(x, cos, sin, Wq, Wk, Wv, Wo):
    import ml_dtypes

    x = np.asarray(x, np.float32)
    cos = np.asarray(cos, np.float32)
    sin = np.asarray(sin, np.float32)
    Wq, Wk, Wv, Wo = (np.asarray(w, np.float32) for w in (Wq, Wk, Wv, Wo))
    cos_bf = cos.astype(ml_dtypes.bfloat16)
    sin_bf = sin.astype(ml_dtypes.bfloat16)

    in_maps = []
    for c in range(NCORES):
        b, hg = divmod(c, 2)
        heads = [HL * hg + i for i in range(HL)]

        def qk_w(W):
            Wsel = np.zeros((EP, D), np.float32)
            for i, g in enumerate(heads):
                Wsel[128 * i : 128 * i + 128] = W[144 * g : 144 * g + 128]
                Wsel[512 + 16 * i : 512 + 16 * i + 16] = W[144 * g + 128 : 144 * g + 144]
            return np.ascontiguousarray(Wsel.T)

        wv_sel = np.concatenate([Wv[144 * g : 144 * g + 144] for g in heads], 0)
        wo_sel = np.concatenate([Wo[:, 144 * g : 144 * g + 144] for g in heads], 1)
        in_maps.append(
            {
                "xT": np.ascontiguousarray(x[b].T).astype(ml_dtypes.bfloat16),
                "wqT": qk_w(Wq).astype(ml_dtypes.bfloat16),
                "wkT": qk_w(Wk).astype(ml_dtypes.bfloat16),
                "wvT": np.ascontiguousarray(wv_sel.T).astype(ml_dtypes.bfloat16),
                "woT": np.ascontiguousarray(wo_sel.T).astype(ml_dtypes.bfloat16),
                "cosN": cos_bf,
                "sinN": sin_bf,
                "identB": np.eye(128, dtype=ml_dtypes.bfloat16),
            }
        )
    return in_maps


def kernel(x, cos, sin, Wq, Wk, Wv, Wo, _trace=False, _trace_kwargs=None):
    from concourse.bass_utils import run_bass_kernel_spmd

    nc = get_nc()
    in_maps = make_in_maps(x, cos, sin, Wq, Wk, Wv, Wo)
    res = run_bass_kernel_spmd(
        nc,
        in_maps,
        list(range(NCORES)),
        trace=_trace,
        **(_trace_kwargs or {}),
    )
    parts = [res.results[c]["out"] for c in range(NCORES)]
    outb = np.stack([parts[2 * b] + parts[2 * b + 1] for b in range(B)])
    if _trace:
        kernel.last_results = res
    return outb.astype(np.float32)
